# revision 8
# baseline (speedup 1.0000x reference)
"""Top-1 MoE layer (Mistral MLP experts, E=2) on 8 Trainium2 cores.

Strategy (dual-expert data-parallel, host does dispatch/combine):
  - Host computes the tiny router (T x E logits, softmax, argmax) in fp64
    and splits EACH expert's tokens evenly across ALL 8 cores, so every
    core carries nA expert-0 slots + nB expert-1 slots (C = nA + nB).
    This rounds capacity at 1/64-token granularity instead of dedicating
    whole cores per expert (C 1032 vs 1040 for the graded seed).
  - Each core receives both experts' weights (pre-tiled, bf16, fully
    contiguous DMA lines) and its packed tokens in a chunk-major layout
    ([P, KT*C] with each chunk's [ki][t] block contiguous) so every x
    DMA line is contiguous on both sides.
  - Device kernel per core (bf16 matmuls, fp32 PSUM): FF processed in
    quarters; h = silu(x@Wg^T) * (x@Wu^T) stays in SBUF; partial down-
    projections accumulate into an SBUF fp32 y buffer; final quarter
    fuses the routing-weight scale. Expert-A chunks use expert-A weight
    tiles, expert-B chunks use expert-B tiles. No collectives.
  - Head latency minimized: first gate tile + first x chunk stream
    per-ki across 3 DMA queues; a few warm-up matmuls keep the PE busy
    (HAM un-throttles) while the first x pieces land. Output DMAs ride
    the gpsimd queue so the weight stream never blocks them.
  - Host scatters per-core outputs back to token order.
"""

import math

import numpy as np
import ml_dtypes

B, S, D, FF, E = 4, 2048, 2048, 8192, 2
T = B * S
P = 128
KT = D // P   # 16 contraction tiles for gate/up
FT = FF // P  # 64 f tiles
DT = D // P   # 16 output-row tiles for down
NQ = 4        # FF quarters
FQ = FT // NQ  # 16 f tiles per quarter
N_CORES = 8
MAX_N = 512   # matmul free-dim / PSUM bank limit (fp32 out)

_nc_cache: dict[tuple, object] = {}

# Last BassKernelResults (for external profiling harnesses).
LAST = None


def _chunks(n):
    """Split n token slots into even chunks <= MAX_N, multiples of 8."""
    if n == 0:
        return []
    k = max(1, math.ceil(n / MAX_N))
    tc = min(MAX_N, ((n + k - 1) // k + 7) // 8 * 8)
    sizes = []
    left = n
    for _ in range(k):
        sizes.append(min(tc, left))
        left -= sizes[-1]
    assert sum(sizes) == n and all(0 < s <= MAX_N for s in sizes)
    return sizes


def _build_nc(nA: int, nB: int):
    """Build + compile the single-core Bass program (SPMD across 8 cores).

    nA/nB = per-core token capacity for expert A / expert B (multiples
    of 8). Every core runs the same program.
    """
    import concourse.mybir as mybir
    import concourse.tile as tile
    from concourse import bacc

    dt = mybir.dt
    nc = bacc.Bacc("TRN2", target_bir_lowering=False, debug=False,
                   num_devices=N_CORES)

    C = nA + nB
    # chunk list: (t0, tn, part) with part 0 = expert A, 1 = expert B
    chunks = [(sum(_chunks(nA)[:i]), s, 0) for i, s in enumerate(_chunks(nA))]
    chunks += [(nA + sum(_chunks(nB)[:i]), s, 1)
               for i, s in enumerate(_chunks(nB))]
    TC = max(s for _, s, _ in chunks)
    parts = sorted({e for _, _, e in chunks})

    # xt[p, KT*t0 + ki*tn + j] = x_packed[t0 + j, ki*128 + p] for each
    # chunk (t0, tn): chunk-major so chunk DMAs are contiguous lines.
    xt_d = nc.dram_tensor("xt", [P, KT * C], dt.bfloat16, kind="ExternalInput")
    # per-expert weight tiles: wg[f, p, ki, m] = w_gate[f*128+m, ki*128+p]
    w_d = {}
    for e in parts:
        sfx = "AB"[e]
        w_d[e] = (
            nc.dram_tensor(f"wg{sfx}", [FT, P, KT, P], dt.bfloat16,
                           kind="ExternalInput"),
            nc.dram_tensor(f"wu{sfx}", [FT, P, KT, P], dt.bfloat16,
                           kind="ExternalInput"),
            nc.dram_tensor(f"wd{sfx}", [DT, NQ, P, FQ, P], dt.bfloat16,
                           kind="ExternalInput"),
        )
    # tw[p, t] = routing weight of token t (same for all p)
    tw_d = nc.dram_tensor("tw", [P, C], dt.float32, kind="ExternalInput")
    # y[do, m, t] = out_packed[t, do*128+m]
    y_d = nc.dram_tensor("y", [DT, P, C], dt.float32, kind="ExternalOutput")

    with tile.TileContext(nc) as tc:
        with (
            tc.tile_pool(name="persist", bufs=1) as pp,
            tc.tile_pool(name="wgwu", bufs=2) as wp,
            tc.tile_pool(name="wdp", bufs=2) as dp,
            tc.tile_pool(name="hbuf", bufs=1) as hp,
            tc.tile_pool(name="stage", bufs=2) as sp,
            tc.tile_pool(name="psum", bufs=2, space="PSUM") as psp,
            tc.tile_pool(name="warm", bufs=1, space="PSUM") as wmp,
        ):
            # weight-stream queues: expert A on sync, expert B on scalar
            # (only SP/Activation/gpsimd engines can issue DMAs)
            wq = {0: nc.sync, 1: nc.scalar}
            # x/tw/head traffic + y output on the remaining capacity
            xq = [nc.scalar, nc.gpsimd]

            # ---- f=0 gate/up tiles for the first part on the sync queue,
            # streamed per-ki so the first LDWEIGHTS can begin after ~32KB.
            e0 = parts[0]
            w0 = {}
            wg_t = wp.tile([P, KT, P], dt.bfloat16, tag=f"wg{e0}")
            for ki in range(KT):
                wq[e0].dma_start(out=wg_t[:, ki : ki + 1, :],
                                 in_=w_d[e0][0][0, :, ki : ki + 1, :])
            wu_t = wp.tile([P, KT, P], dt.bfloat16, tag=f"wu{e0}")
            wq[e0].dma_start(out=wu_t[:], in_=w_d[e0][1][0])
            w0[e0] = (wg_t, wu_t)

            # ---- PE warm-up: a few matmuls on the first weight slice so
            # the HAM clock-gate opens while x still streams in.
            warm_ps = wmp.tile([P, P], dt.float32, tag="warm")
            for _ in range(6):
                nc.tensor.matmul(warm_ps[:], wg_t[:, 0:1, :], wg_t[:, 0:1, :],
                                 start=True, stop=True)

            # ---- x loads: first chunk per-ki round-robin across queues,
            # later chunks whole (contiguous [P, KT*tn] lines).
            xt = pp.tile([P, KT * C], dt.bfloat16)
            qi = 0
            for ci, (t0, tn, e) in enumerate(chunks):
                base = KT * t0
                if ci == 0:
                    for ki in range(KT):
                        sl = slice(base + ki * tn, base + (ki + 1) * tn)
                        xq[qi % 2].dma_start(out=xt[:, sl], in_=xt_d[:, sl])
                        qi += 1
                else:
                    sl = slice(base, base + KT * tn)
                    xq[qi % 2].dma_start(out=xt[:, sl], in_=xt_d[:, sl])
                    qi += 1
            tw = pp.tile([P, C], dt.float32)
            nc.gpsimd.dma_start(out=tw[:], in_=tw_d[:])

            # ---- f=0 tiles for the second part (after the x pieces on
            # the scalar queue so they don't delay the first matmuls).
            for e in parts[1:]:
                wg_t = wp.tile([P, KT, P], dt.bfloat16, tag=f"wg{e}")
                wq[e].dma_start(out=wg_t[:], in_=w_d[e][0][0])
                wu_t = wp.tile([P, KT, P], dt.bfloat16, tag=f"wu{e}")
                wq[e].dma_start(out=wu_t[:], in_=w_d[e][1][0])
                w0[e] = (wg_t, wu_t)

            h = hp.tile([P, FQ, C], dt.bfloat16)
            y_acc = pp.tile([P, DT, C], dt.float32)

            def xsl(t0, tn, ki):
                return xt[:, KT * t0 + ki * tn : KT * t0 + (ki + 1) * tn]

            for q in range(NQ):
                # phase A: h = silu(x @ Wg^T) * (x @ Wu^T) for this quarter
                for fl in range(FQ):
                    f = q * FQ + fl
                    wt = {}
                    for e in parts:
                        if f == 0:
                            wt[e] = w0[e]
                        else:
                            wg_t = wp.tile([P, KT, P], dt.bfloat16,
                                           tag=f"wg{e}")
                            wq[e].dma_start(out=wg_t[:], in_=w_d[e][0][f])
                            wu_t = wp.tile([P, KT, P], dt.bfloat16,
                                           tag=f"wu{e}")
                            wq[e].dma_start(out=wu_t[:], in_=w_d[e][1][f])
                            wt[e] = (wg_t, wu_t)
                    for ci, (t0, tn, e) in enumerate(chunks):
                        wg_t, wu_t = wt[e]
                        g_ps = psp.tile([P, TC], dt.float32, tag="g")
                        u_ps = psp.tile([P, TC], dt.float32, tag="u")
                        for ki in range(KT):
                            nc.tensor.matmul(
                                g_ps[:, :tn],
                                wg_t[:, ki : ki + 1, :],
                                xsl(t0, tn, ki),
                                start=(ki == 0),
                                stop=(ki == KT - 1),
                            )
                        for ki in range(KT):
                            nc.tensor.matmul(
                                u_ps[:, :tn],
                                wu_t[:, ki : ki + 1, :],
                                xsl(t0, tn, ki),
                                start=(ki == 0),
                                stop=(ki == KT - 1),
                            )
                        hsl = slice(t0, t0 + tn)
                        sg = sp.tile([P, TC], dt.float32, tag="sg")
                        nc.scalar.activation(
                            sg[:, :tn], g_ps[:, :tn],
                            mybir.ActivationFunctionType.Silu,
                        )
                        nc.vector.tensor_mul(
                            h[:, fl, hsl], sg[:, :tn], u_ps[:, :tn]
                        )
                # phase B: y_acc += h @ Wd^T (this quarter's partial)
                for do in range(DT):
                    wdt = {}
                    for e in parts:
                        wd_t = dp.tile([P, FQ, P], dt.bfloat16, tag=f"wd{e}")
                        wq[e].dma_start(out=wd_t[:], in_=w_d[e][2][do, q])
                        wdt[e] = wd_t
                    for ci, (t0, tn, e) in enumerate(chunks):
                        tsl = slice(t0, t0 + tn)
                        hsl = tsl
                        y_ps = psp.tile([P, TC], dt.float32, tag="y")
                        for fl in range(FQ):
                            nc.tensor.matmul(
                                y_ps[:, :tn],
                                wdt[e][:, fl : fl + 1, :],
                                h[:, fl, hsl],
                                start=(fl == 0),
                                stop=(fl == FQ - 1),
                            )
                        if q == 0:
                            nc.vector.tensor_copy(
                                y_acc[:, do, tsl], y_ps[:, :tn]
                            )
                        elif q < NQ - 1:
                            nc.vector.tensor_add(
                                y_acc[:, do, tsl], y_acc[:, do, tsl],
                                y_ps[:, :tn],
                            )
                        else:
                            y_sb = sp.tile([P, TC], dt.float32, tag="yo")
                            nc.vector.tensor_add(
                                y_sb[:, :tn], y_acc[:, do, tsl],
                                y_ps[:, :tn],
                            )
                            nc.vector.tensor_mul(
                                y_sb[:, :tn], y_sb[:, :tn], tw[:, tsl]
                            )
                            nc.gpsimd.dma_start(
                                out=y_d[do, :, tsl], in_=y_sb[:, :tn]
                            )

    nc.compile()
    return nc


def _tile_w_in(w_t):
    """[D, FF] (already transposed) -> [FF/P, P, D/P, P] contiguous bf16."""
    # out[f, p, ki, m] = w_t[ki*128+p, f*128+m]
    r = w_t.reshape(KT, P, FT, P).transpose(2, 1, 0, 3)
    return np.ascontiguousarray(r, dtype=ml_dtypes.bfloat16)


def _tile_w_down(w):
    """w_down [D, FF] -> [D/P, NQ, P, FQ, P] contiguous bf16.

    out[do, q, p, fl, m] = w[do*128+m, (q*FQ+fl)*128+p]
    """
    r = w.reshape(DT, P, NQ, FQ, P).transpose(0, 2, 4, 3, 1)
    return np.ascontiguousarray(r, dtype=ml_dtypes.bfloat16)


def _pack_x(x_slots, chunk_list, C):
    """x_slots [C, D] fp32 (padded rows zero) -> [P, KT*C] chunk-major bf16."""
    xt = np.zeros((P, KT * C), dtype=ml_dtypes.bfloat16)
    xb = x_slots.astype(ml_dtypes.bfloat16)
    for t0, tn, _ in chunk_list:
        blk = xb[t0 : t0 + tn].T.reshape(KT, P, tn)  # [ki, p, t]
        xt[:, KT * t0 : KT * (t0 + tn)] = (
            blk.transpose(1, 0, 2).reshape(P, KT * tn)
        )
    return xt


def kernel(hidden_states, gate_w, w_gate, w_up, w_down):
    from concourse.bass_utils import run_bass_kernel_spmd

    hidden_states = np.asarray(hidden_states)
    gate_w = np.asarray(gate_w)
    w_gate = np.asarray(w_gate)
    w_up = np.asarray(w_up)
    w_down = np.asarray(w_down)

    x = hidden_states.reshape(T, D)

    # --- router (tiny: T x E) on host, fp64 for stable argmax ---
    logits = x.astype(np.float64) @ gate_w.astype(np.float64).T  # [T, E]
    m = logits.max(axis=1, keepdims=True)
    p = np.exp(logits - m)
    p /= p.sum(axis=1, keepdims=True)
    sel = np.argmax(p, axis=1)  # [T]
    top_w = p[np.arange(T), sel].astype(np.float32)  # [T]

    # --- dispatch: split BOTH experts' tokens across all 8 cores ---
    idx_e = [np.nonzero(sel == e)[0] for e in range(E)]
    L = [len(idx_e[0]), len(idx_e[1])]
    cap = [((math.ceil(l / N_CORES) + 7) // 8) * 8 for l in L]
    nA, nB = cap
    C = nA + nB

    nc = _nc_cache.get((nA, nB))
    if nc is None:
        nc = _build_nc(nA, nB)
        _nc_cache[(nA, nB)] = nc

    chunk_list = [(sum(_chunks(nA)[:i]), s, 0)
                  for i, s in enumerate(_chunks(nA))]
    chunk_list += [(nA + sum(_chunks(nB)[:i]), s, 1)
                   for i, s in enumerate(_chunks(nB))]

    # --- per-expert weight tiling (shared across cores) ---
    wg_tiled = [_tile_w_in(w_gate[e].T) for e in range(E)]
    wu_tiled = [_tile_w_in(w_up[e].T) for e in range(E)]
    wd_tiled = [_tile_w_down(w_down[e]) for e in range(E)]

    # per-core token slot assignment: core c takes an even share of each
    # expert's index list
    core_tok = []       # per core: array of token ids in slot order
    core_slot0 = []     # per core: slot index of each token (for combine)
    in_maps = []
    for c in range(N_CORES):
        ids_parts = []
        slots_parts = []
        x_slots = np.zeros((C, D), dtype=np.float32)
        tw = np.zeros((P, C), dtype=np.float32)
        for e, (n_e, off) in enumerate(zip(cap, (0, nA))):
            ids_all = idx_e[e]
            le = len(ids_all)
            base = (le * c) // N_CORES
            end = (le * (c + 1)) // N_CORES
            ids = ids_all[base:end]
            n = len(ids)
            assert n <= n_e
            if n:
                x_slots[off : off + n] = x[ids]
                tw[:, off : off + n] = top_w[ids][None, :]
                ids_parts.append(ids)
                slots_parts.append(np.arange(off, off + n))
        ids_c = (np.concatenate(ids_parts) if ids_parts
                 else np.zeros(0, dtype=np.int64))
        slots_c = (np.concatenate(slots_parts) if slots_parts
                   else np.zeros(0, dtype=np.int64))
        core_tok.append(ids_c)
        core_slot0.append(slots_c)
        im = {
            "xt": _pack_x(x_slots, chunk_list, C),
            "tw": tw,
        }
        for e in range(E):
            if not cap[e]:
                continue
            sfx = "AB"[e]
            im[f"wg{sfx}"] = wg_tiled[e]
            im[f"wu{sfx}"] = wu_tiled[e]
            im[f"wd{sfx}"] = wd_tiled[e]
        in_maps.append(im)

    res = run_bass_kernel_spmd(nc, in_maps, list(range(N_CORES)))
    global LAST
    LAST = res

    # --- combine ---
    out = np.zeros((T, D), dtype=np.float32)
    for c in range(N_CORES):
        ids = core_tok[c]
        if not len(ids):
            continue
        y = res.results[c]["y"]  # [DT, P, C]
        out[ids] = y.reshape(D, C)[:, core_slot0[c]].T
    return out.reshape(B, S, D)


# revision 13
# speedup vs baseline: 1.0098x; 1.0098x over previous
"""Top-1 MoE layer (Mistral MLP experts, E=2) on 8 Trainium2 cores.

Strategy (dual-expert data-parallel, host does dispatch/combine):
  - Host computes the tiny router (T x E logits, softmax, argmax) in fp64
    and splits EACH expert's tokens evenly across ALL 8 cores, so every
    core carries nA expert-0 slots + nB expert-1 slots (C = nA + nB).
    This rounds capacity at 1/64-token granularity instead of dedicating
    whole cores per expert (C 1032 vs 1040 for the graded seed).
  - Each core receives both experts' weights (pre-tiled, bf16, fully
    contiguous DMA lines) and its packed tokens in a chunk-major layout
    ([P, KT*C] with each chunk's [ki][t] block contiguous) so every x
    DMA line is contiguous on both sides.
  - Device kernel per core (bf16 matmuls, fp32 PSUM): FF processed in
    quarters; h = silu(x@Wg^T) * (x@Wu^T) stays in SBUF; partial down-
    projections accumulate into an SBUF fp32 y buffer; final quarter
    fuses the routing-weight scale. Expert-A chunks use expert-A weight
    tiles, expert-B chunks use expert-B tiles. No collectives.
  - Head latency minimized: first gate tile + first x chunk stream
    per-ki across 3 DMA queues; a few warm-up matmuls keep the PE busy
    (HAM un-throttles) while the first x pieces land. Output DMAs ride
    the gpsimd queue so the weight stream never blocks them.
  - Host scatters per-core outputs back to token order.
"""

import math

import numpy as np
import ml_dtypes

B, S, D, FF, E = 4, 2048, 2048, 8192, 2
T = B * S
P = 128
KT = D // P   # 16 contraction tiles for gate/up
FT = FF // P  # 64 f tiles
DT = D // P   # 16 output-row tiles for down
NQ = 4        # FF quarters
FQ = FT // NQ  # 16 f tiles per quarter
N_CORES = 8
MAX_N = 512   # matmul free-dim / PSUM bank limit (fp32 out)

_nc_cache: dict[tuple, object] = {}

# Last BassKernelResults (for external profiling harnesses).
LAST = None


def _chunks(n):
    """Split n token slots into even chunks <= MAX_N, multiples of 8."""
    if n == 0:
        return []
    k = max(1, math.ceil(n / MAX_N))
    tc = min(MAX_N, ((n + k - 1) // k + 7) // 8 * 8)
    sizes = []
    left = n
    for _ in range(k):
        sizes.append(min(tc, left))
        left -= sizes[-1]
    assert sum(sizes) == n and all(0 < s <= MAX_N for s in sizes)
    return sizes


def _build_nc(nA: int, nB: int):
    """Build + compile the single-core Bass program (SPMD across 8 cores).

    nA/nB = per-core token capacity for expert A / expert B (multiples
    of 8). Every core runs the same program.
    """
    import concourse.mybir as mybir
    import concourse.tile as tile
    from concourse import bacc

    dt = mybir.dt
    nc = bacc.Bacc("TRN2", target_bir_lowering=False, debug=False,
                   num_devices=N_CORES)

    C = nA + nB
    # chunk list: (t0, tn, part) with part 0 = expert A, 1 = expert B
    chunks = [(sum(_chunks(nA)[:i]), s, 0) for i, s in enumerate(_chunks(nA))]
    chunks += [(nA + sum(_chunks(nB)[:i]), s, 1)
               for i, s in enumerate(_chunks(nB))]
    TC = max(s for _, s, _ in chunks)
    parts = sorted({e for _, _, e in chunks})

    # xt[p, KT*t0 + ki*tn + j] = x_packed[t0 + j, ki*128 + p] for each
    # chunk (t0, tn): chunk-major so chunk DMAs are contiguous lines.
    xt_d = nc.dram_tensor("xt", [P, KT * C], dt.bfloat16, kind="ExternalInput")
    # per-expert weight tiles: wg[f, p, ki, m] = w_gate[f*128+m, ki*128+p]
    w_d = {}
    for e in parts:
        sfx = "AB"[e]
        w_d[e] = (
            nc.dram_tensor(f"wg{sfx}", [FT, P, KT, P], dt.bfloat16,
                           kind="ExternalInput"),
            nc.dram_tensor(f"wu{sfx}", [FT, P, KT, P], dt.bfloat16,
                           kind="ExternalInput"),
            nc.dram_tensor(f"wd{sfx}", [DT, NQ, P, FQ, P], dt.bfloat16,
                           kind="ExternalInput"),
        )
    # tw[p, t] = routing weight of token t (same for all p)
    tw_d = nc.dram_tensor("tw", [P, C], dt.float32, kind="ExternalInput")
    # y[do, m, t] = out_packed[t, do*128+m]
    y_d = nc.dram_tensor("y", [DT, P, C], dt.float32, kind="ExternalOutput")

    with tile.TileContext(nc) as tc:
        with (
            tc.tile_pool(name="persist", bufs=1) as pp,
            tc.tile_pool(name="wgwu", bufs=2) as wp,
            tc.tile_pool(name="wdp", bufs=2) as dp,
            tc.tile_pool(name="hbuf", bufs=1) as hp,
            tc.tile_pool(name="stage", bufs=2) as sp,
            tc.tile_pool(name="yout", bufs=3) as yp,
            tc.tile_pool(name="psum", bufs=2, space="PSUM") as psp,
            tc.tile_pool(name="psumy", bufs=4, space="PSUM") as pyp,
        ):
            # ALL weights stream on the sync queue; x/tw ride scalar +
            # gpsimd; y output rides gpsimd. Strict separation so a
            # blocked weight-prefetch trigger (semaphore not yet met)
            # never head-of-line-blocks x or y traffic.
            wq = {0: nc.sync, 1: nc.sync}
            xq = [nc.scalar, nc.gpsimd]

            # ---- f=0 gate/up tiles for the first part on the sync queue,
            # streamed per-ki so the first LDWEIGHTS can begin after ~32KB.
            e0 = parts[0]
            w0 = {}
            wg_t = wp.tile([P, KT, P], dt.bfloat16, tag=f"wg{e0}")
            for ki in range(KT):
                wq[e0].dma_start(out=wg_t[:, ki : ki + 1, :],
                                 in_=w_d[e0][0][0, :, ki : ki + 1, :])
            wu_t = wp.tile([P, KT, P], dt.bfloat16, tag=f"wu{e0}")
            wq[e0].dma_start(out=wu_t[:], in_=w_d[e0][1][0])
            w0[e0] = (wg_t, wu_t)

            # ---- PE warm-up: a few matmuls on the first weight slice so
            # the HAM clock-gate opens while x still streams in.
            warm_ps = psp.tile([P, TC], dt.float32, tag="g")
            for _ in range(6):
                nc.tensor.matmul(warm_ps[:, :P], wg_t[:, 0:1, :],
                                 wg_t[:, 0:1, :], start=True, stop=True)

            # ---- x loads: first chunk per-ki round-robin across queues,
            # later chunks whole (contiguous [P, KT*tn] lines).
            xt = pp.tile([P, KT * C], dt.bfloat16)
            qi = 0
            for ci, (t0, tn, e) in enumerate(chunks):
                base = KT * t0
                if ci == 0:
                    for ki in range(KT):
                        sl = slice(base + ki * tn, base + (ki + 1) * tn)
                        xq[qi % 2].dma_start(out=xt[:, sl], in_=xt_d[:, sl])
                        qi += 1
                else:
                    sl = slice(base, base + KT * tn)
                    xq[qi % 2].dma_start(out=xt[:, sl], in_=xt_d[:, sl])
                    qi += 1
            tw = pp.tile([P, C], dt.float32)
            nc.scalar.dma_start(out=tw[:], in_=tw_d[:])

            # ---- f=0 tiles for the second part (after the x pieces on
            # the scalar queue so they don't delay the first matmuls).
            for e in parts[1:]:
                wg_t = wp.tile([P, KT, P], dt.bfloat16, tag=f"wg{e}")
                wq[e].dma_start(out=wg_t[:], in_=w_d[e][0][0])
                wu_t = wp.tile([P, KT, P], dt.bfloat16, tag=f"wu{e}")
                wq[e].dma_start(out=wu_t[:], in_=w_d[e][1][0])
                w0[e] = (wg_t, wu_t)

            h = hp.tile([P, FQ, C], dt.bfloat16)
            y_acc = pp.tile([P, DT, C], dt.float32)

            def xsl(t0, tn, ki):
                return xt[:, KT * t0 + ki * tn : KT * t0 + (ki + 1) * tn]

            for q in range(NQ):
                # phase A: h = silu(x @ Wg^T) * (x @ Wu^T) for this quarter
                for fl in range(FQ):
                    f = q * FQ + fl
                    wt = {}
                    for e in parts:
                        if f == 0:
                            wt[e] = w0[e]
                        else:
                            wg_t = wp.tile([P, KT, P], dt.bfloat16,
                                           tag=f"wg{e}")
                            wq[e].dma_start(out=wg_t[:], in_=w_d[e][0][f])
                            wu_t = wp.tile([P, KT, P], dt.bfloat16,
                                           tag=f"wu{e}")
                            wq[e].dma_start(out=wu_t[:], in_=w_d[e][1][f])
                            wt[e] = (wg_t, wu_t)
                    for ci, (t0, tn, e) in enumerate(chunks):
                        wg_t, wu_t = wt[e]
                        g_ps = psp.tile([P, TC], dt.float32, tag="g")
                        u_ps = psp.tile([P, TC], dt.float32, tag="u")
                        for ki in range(KT):
                            nc.tensor.matmul(
                                g_ps[:, :tn],
                                wg_t[:, ki : ki + 1, :],
                                xsl(t0, tn, ki),
                                start=(ki == 0),
                                stop=(ki == KT - 1),
                            )
                        for ki in range(KT):
                            nc.tensor.matmul(
                                u_ps[:, :tn],
                                wu_t[:, ki : ki + 1, :],
                                xsl(t0, tn, ki),
                                start=(ki == 0),
                                stop=(ki == KT - 1),
                            )
                        hsl = slice(t0, t0 + tn)
                        sg = sp.tile([P, TC], dt.float32, tag="sg")
                        nc.scalar.activation(
                            sg[:, :tn], g_ps[:, :tn],
                            mybir.ActivationFunctionType.Silu,
                        )
                        nc.vector.tensor_mul(
                            h[:, fl, hsl], sg[:, :tn], u_ps[:, :tn]
                        )
                # phase B: y_acc += h @ Wd^T (this quarter's partial)
                for do in range(DT):
                    wdt = {}
                    for e in parts:
                        wd_t = dp.tile([P, FQ, P], dt.bfloat16, tag=f"wd{e}")
                        wq[e].dma_start(out=wd_t[:], in_=w_d[e][2][do, q])
                        wdt[e] = wd_t
                    for ci, (t0, tn, e) in enumerate(chunks):
                        tsl = slice(t0, t0 + tn)
                        hsl = tsl
                        y_ps = pyp.tile([P, TC], dt.float32, tag="y")
                        for fl in range(FQ):
                            nc.tensor.matmul(
                                y_ps[:, :tn],
                                wdt[e][:, fl : fl + 1, :],
                                h[:, fl, hsl],
                                start=(fl == 0),
                                stop=(fl == FQ - 1),
                            )
                        if q == 0:
                            nc.vector.tensor_copy(
                                y_acc[:, do, tsl], y_ps[:, :tn]
                            )
                        elif q < NQ - 1:
                            nc.vector.tensor_add(
                                y_acc[:, do, tsl], y_acc[:, do, tsl],
                                y_ps[:, :tn],
                            )
                        else:
                            y_sb = yp.tile([P, TC], dt.float32, tag="yo")
                            nc.vector.tensor_add(
                                y_sb[:, :tn], y_acc[:, do, tsl],
                                y_ps[:, :tn],
                            )
                            nc.vector.tensor_mul(
                                y_sb[:, :tn], y_sb[:, :tn], tw[:, tsl]
                            )
                            nc.gpsimd.dma_start(
                                out=y_d[do, :, tsl], in_=y_sb[:, :tn]
                            )

    nc.compile()
    return nc


def _tile_w_in(w_t):
    """[D, FF] (already transposed) -> [FF/P, P, D/P, P] contiguous bf16."""
    # out[f, p, ki, m] = w_t[ki*128+p, f*128+m]
    r = w_t.reshape(KT, P, FT, P).transpose(2, 1, 0, 3)
    return np.ascontiguousarray(r, dtype=ml_dtypes.bfloat16)


def _tile_w_down(w):
    """w_down [D, FF] -> [D/P, NQ, P, FQ, P] contiguous bf16.

    out[do, q, p, fl, m] = w[do*128+m, (q*FQ+fl)*128+p]
    """
    r = w.reshape(DT, P, NQ, FQ, P).transpose(0, 2, 4, 3, 1)
    return np.ascontiguousarray(r, dtype=ml_dtypes.bfloat16)


def _pack_x(x_slots, chunk_list, C):
    """x_slots [C, D] fp32 (padded rows zero) -> [P, KT*C] chunk-major bf16."""
    xt = np.zeros((P, KT * C), dtype=ml_dtypes.bfloat16)
    xb = x_slots.astype(ml_dtypes.bfloat16)
    for t0, tn, _ in chunk_list:
        blk = xb[t0 : t0 + tn].T.reshape(KT, P, tn)  # [ki, p, t]
        xt[:, KT * t0 : KT * (t0 + tn)] = (
            blk.transpose(1, 0, 2).reshape(P, KT * tn)
        )
    return xt


def kernel(hidden_states, gate_w, w_gate, w_up, w_down):
    from concourse.bass_utils import run_bass_kernel_spmd

    hidden_states = np.asarray(hidden_states)
    gate_w = np.asarray(gate_w)
    w_gate = np.asarray(w_gate)
    w_up = np.asarray(w_up)
    w_down = np.asarray(w_down)

    x = hidden_states.reshape(T, D)

    # --- router (tiny: T x E) on host, fp64 for stable argmax ---
    logits = x.astype(np.float64) @ gate_w.astype(np.float64).T  # [T, E]
    m = logits.max(axis=1, keepdims=True)
    p = np.exp(logits - m)
    p /= p.sum(axis=1, keepdims=True)
    sel = np.argmax(p, axis=1)  # [T]
    top_w = p[np.arange(T), sel].astype(np.float32)  # [T]

    # --- dispatch: split BOTH experts' tokens across all 8 cores ---
    idx_e = [np.nonzero(sel == e)[0] for e in range(E)]
    L = [len(idx_e[0]), len(idx_e[1])]
    cap = [((math.ceil(l / N_CORES) + 7) // 8) * 8 for l in L]
    nA, nB = cap
    C = nA + nB

    nc = _nc_cache.get((nA, nB))
    if nc is None:
        nc = _build_nc(nA, nB)
        _nc_cache[(nA, nB)] = nc

    chunk_list = [(sum(_chunks(nA)[:i]), s, 0)
                  for i, s in enumerate(_chunks(nA))]
    chunk_list += [(nA + sum(_chunks(nB)[:i]), s, 1)
                   for i, s in enumerate(_chunks(nB))]

    # --- per-expert weight tiling (shared across cores) ---
    wg_tiled = [_tile_w_in(w_gate[e].T) for e in range(E)]
    wu_tiled = [_tile_w_in(w_up[e].T) for e in range(E)]
    wd_tiled = [_tile_w_down(w_down[e]) for e in range(E)]

    # per-core token slot assignment: core c takes an even share of each
    # expert's index list
    core_tok = []       # per core: array of token ids in slot order
    core_slot0 = []     # per core: slot index of each token (for combine)
    in_maps = []
    for c in range(N_CORES):
        ids_parts = []
        slots_parts = []
        x_slots = np.zeros((C, D), dtype=np.float32)
        tw = np.zeros((P, C), dtype=np.float32)
        for e, (n_e, off) in enumerate(zip(cap, (0, nA))):
            ids_all = idx_e[e]
            le = len(ids_all)
            base = (le * c) // N_CORES
            end = (le * (c + 1)) // N_CORES
            ids = ids_all[base:end]
            n = len(ids)
            assert n <= n_e
            if n:
                x_slots[off : off + n] = x[ids]
                tw[:, off : off + n] = top_w[ids][None, :]
                ids_parts.append(ids)
                slots_parts.append(np.arange(off, off + n))
        ids_c = (np.concatenate(ids_parts) if ids_parts
                 else np.zeros(0, dtype=np.int64))
        slots_c = (np.concatenate(slots_parts) if slots_parts
                   else np.zeros(0, dtype=np.int64))
        core_tok.append(ids_c)
        core_slot0.append(slots_c)
        im = {
            "xt": _pack_x(x_slots, chunk_list, C),
            "tw": tw,
        }
        for e in range(E):
            if not cap[e]:
                continue
            sfx = "AB"[e]
            im[f"wg{sfx}"] = wg_tiled[e]
            im[f"wu{sfx}"] = wu_tiled[e]
            im[f"wd{sfx}"] = wd_tiled[e]
        in_maps.append(im)

    res = run_bass_kernel_spmd(nc, in_maps, list(range(N_CORES)))
    global LAST
    LAST = res

    # --- combine ---
    out = np.zeros((T, D), dtype=np.float32)
    for c in range(N_CORES):
        ids = core_tok[c]
        if not len(ids):
            continue
        y = res.results[c]["y"]  # [DT, P, C]
        out[ids] = y.reshape(D, C)[:, core_slot0[c]].T
    return out.reshape(B, S, D)
